# revision 21
# baseline (speedup 1.0000x reference)
"""MCInfoNCE loss on 8 Trainium2 NeuronCores (Bass/Tile).

Sharding: data-parallel over the query axis of the [S, B, B] score tensor.
Each core owns B/8 = 512 query rows and sees all B negative keys
(replicated key side, i.e. the "all-gather" of standard distributed
InfoNCE is realized by handing every core the full key tensors). The key
tensors handed to core c are rotated by c*512 rows so that each core's
own positive diagonal block sits at local column block 0 -- this keeps
the compiled program identical across cores (pure SPMD, no partition id).

The vMF proposal samples around e1 (Wood's accept-reject scheme) depend
only on the fixed RNG key(42), never on the inputs, so they are
reproduced once on the host with jax-CPU and streamed to the device as
constants. Everything input-dependent runs on device: normalization,
Householder reflection, the [S,B,B] score matmul, the
logsumexp-over-keys, the logsumexp-over-samples, and the final mean.

Device pipeline per core:
  1. prep:  sumsq(z) -> rsqrt (Newton on DVE) -> u = z*rn - e1,
            c2 = 2/(1-mu0)   (Householder with unnormalized u)
  2. per sample s: dot = <e, u> (DVE ttr), w = u*dot*c2 (DVE ts),
            srow = e - w (DVE tt, bf16), then one xbar DMA transpose
            into the [D, S*B] key matrix / [D, S*512] query matrix.
  3. scores: PE matmuls (bf16, K=D=128) into [128, 2048] PSUM chunks;
            diagonal extracted via identity ttr; ScalarE computes
            exp(K_POS*x - 20) in place with row-accumulate.
  4. tail:  ln / exp / ln stack on [128, 32] buffers, partition
            all-reduce, one scalar DMA out per core. Host sums 8
            partials and divides by B.
"""

import numpy as np

S, B, D = 8, 4096, 128
NCORES = 8
BSH = B // NCORES          # 512 query rows per core
RT = BSH // 128            # 4 row tiles per core
JT = B // 128              # 32 key tiles
KAPPA, K_POS, ROUNDS = 32.0, 20.0, 8
SHIFT = 20.0               # fixed logsumexp shift (scores <= K_POS = 20)
LNB = float(np.log(B))
LNS = float(np.log(S))
CHUNK = 2048               # PSUM chunk (4 banks); 2 chunks in flight
NCHUNK = B // CHUNK
NEWTON_ITERS = 4

_cache = {}


# --------------------------------------------------------------------------
# Host-side constants: vMF samples around e1 (input independent, fixed seed)
# --------------------------------------------------------------------------

def _e1_samples():
    """Reproduce the reference's vMF sampling up to (but excluding) the
    Householder reflection: returns two [S, B, D] float32 arrays."""
    import jax
    import jax.numpy as jnp

    cpu = jax.devices("cpu")[0]

    def sample(key, n_samples, Bb, Dd):
        m = float(Dd)
        k = KAPPA
        sq = jnp.sqrt(4.0 * k * k + (m - 1.0) ** 2)
        b = (-2.0 * k + sq) / (m - 1.0)
        a = (m - 1.0 + 2.0 * k + sq) / 4.0
        d = 4.0 * a * b / (1.0 + b) - (m - 1.0) * jnp.log(m - 1.0)

        kb, ku, kv = jax.random.split(key, 3)
        eps = jax.random.beta(kb, (m - 1.0) / 2.0, (m - 1.0) / 2.0,
                              (ROUNDS, n_samples, Bb))
        u = jax.random.uniform(ku, (ROUNDS, n_samples, Bb))
        denom = 1.0 - (1.0 - b) * eps
        w_prop = (1.0 - (1.0 + b) * eps) / denom
        t = 2.0 * a * b / denom
        accept = (m - 1.0) * jnp.log(t) - t + d >= jnp.log(u)
        first = jnp.argmax(accept, axis=0)
        w = jnp.take_along_axis(w_prop, first[None], axis=0)[0]

        v = jax.random.normal(kv, (n_samples, Bb, Dd - 1))
        v = v / jnp.linalg.norm(v, axis=-1, keepdims=True)
        z = jnp.concatenate(
            [w[..., None], jnp.sqrt(jnp.clip(1.0 - w * w, 0.0))[..., None] * v],
            axis=-1)
        return np.asarray(z, dtype=np.float32)

    with jax.default_device(cpu):
        ka, kb = jax.random.split(jax.random.key(42))
        z1e = sample(ka, S, B, D)
        z2e = sample(kb, S, B, D)
    return z1e, z2e


def _host_consts():
    """Per-core constant input arrays (cached)."""
    if "consts" in _cache:
        return _cache["consts"]
    import ml_dtypes
    bf16 = ml_dtypes.bfloat16
    z1e, z2e = _e1_samples()
    e1_percore = []
    e2_percore = []
    for c in range(NCORES):
        # queries: own 512 rows, [S, 128, RT, 128]
        e1c = z1e[:, c * BSH:(c + 1) * BSH, :]
        e1c = e1c.reshape(S, RT, 128, 128).transpose(0, 2, 1, 3)
        e1_percore.append(np.ascontiguousarray(e1c).astype(bf16))
        # keys: full set rotated by c*512, [S, 128, JT, 128]
        e2c = np.roll(z2e, -c * BSH, axis=1)
        e2c = e2c.reshape(S, JT, 128, 128).transpose(0, 2, 1, 3)
        e2_percore.append(np.ascontiguousarray(e2c).astype(bf16))
    iden = np.eye(128, dtype=np.float32)
    _cache["consts"] = (e1_percore, e2_percore, iden)
    return _cache["consts"]


# --------------------------------------------------------------------------
# Device program
# --------------------------------------------------------------------------

def _build_program(stage=4):
    """stage: 1=loads+prep, 2=+householder/transpose, 3=+scores/exp, 4=full."""
    key = ("prog", stage)
    if key in _cache:
        return _cache[key]
    import concourse.bass as bass
    from concourse import bacc, mybir
    from concourse.tile import TileContext

    f32 = mybir.dt.float32
    bf16 = mybir.dt.bfloat16
    i32 = mybir.dt.int32
    A = mybir.AluOpType
    AF = mybir.ActivationFunctionType
    AX = mybir.AxisListType

    nc = bacc.Bacc()
    Z2 = nc.declare_dram_parameter("z2r", [128, JT, 128], f32, isOutput=False)
    Z1 = nc.declare_dram_parameter("z1s", [128, RT, 128], f32, isOutput=False)
    E2 = nc.declare_dram_parameter("e2", [S, 128, JT, 128], bf16, isOutput=False)
    E1 = nc.declare_dram_parameter("e1", [S, 128, RT, 128], bf16, isOutput=False)
    IDN = nc.declare_dram_parameter("iden", [128, 128], f32, isOutput=False)
    OUT = nc.declare_dram_parameter("out", [128, 1], f32, isOutput=True)

    with TileContext(nc) as tc:
        with (
            tc.tile_pool(name="consts", bufs=1) as cpool,
            tc.tile_pool(name="stream", bufs=2) as spool,
            tc.tile_pool(name="scr", bufs=4) as scrpool,
            tc.tile_pool(name="psum", bufs=2, space="PSUM") as ppool,
        ):
            z2sb = cpool.tile([128, JT, 128], f32)
            z1sb = cpool.tile([128, RT, 128], f32)
            iden = cpool.tile([128, 128], f32)
            u2 = cpool.tile([128, JT, 128], bf16)
            u1 = cpool.tile([128, RT, 128], bf16)
            s2T = cpool.tile([128, S, B], bf16)
            s1T = cpool.tile([128, S, BSH], bf16)
            ss2 = cpool.tile([128, JT], f32)
            rn2 = cpool.tile([128, JT], f32)
            c22 = cpool.tile([128, JT], f32)
            ss1 = cpool.tile([128, RT], f32)
            rn1 = cpool.tile([128, RT], f32)
            c21 = cpool.tile([128, RT], f32)
            SUMS2 = cpool.tile([128, RT * S * NCHUNK], f32)
            DIAG = cpool.tile([128, RT * S], f32)
            SUMS = cpool.tile([128, RT * S], f32)
            LSE = cpool.tile([128, RT * S], f32)
            RATIO = cpool.tile([128, RT * S], f32)
            TEXP = cpool.tile([128, RT * S], f32)
            T4 = cpool.tile([128, RT], f32)
            LG = cpool.tile([128, RT], f32)
            LI = cpool.tile([128, RT], f32)
            LIC = cpool.tile([128, 1], f32)
            bm_shift = cpool.tile([128, 1], f32)
            bm_lnb = cpool.tile([128, 1], f32)
            nc.vector.memset(bm_shift, -SHIFT)
            nc.vector.memset(bm_lnb, -LNB)

            nc.sync.dma_start(out=z2sb, in_=Z2[:])
            nc.sync.dma_start(out=z1sb, in_=Z1[:])
            nc.sync.dma_start(out=iden, in_=IDN[:])

            # ---- prep: rn = 1/||z||  (Newton rsqrt on DVE), u, c2 ----
            def prep(zsb, nt, ss, rn, c2, u):
                for t in range(nt):
                    scr = scrpool.tile([128, 128], f32, tag="ttscr")
                    nc.scalar.activation(
                        out=scr, in_=zsb[:, t, :], func=AF.Square,
                        accum_out=ss[:, t:t + 1])
                # rsqrt seed via int bit trick: y = 0x5f3759df - (bits >> 1)
                tmpi = scrpool.tile([128, nt], i32, tag="rsq_i")
                nc.vector.tensor_scalar(
                    out=tmpi, in0=ss[:].bitcast(i32), scalar1=1, scalar2=None,
                    op0=A.logical_shift_right)
                nc.vector.tensor_scalar(
                    out=rn[:].bitcast(i32), in0=tmpi, scalar1=-1,
                    scalar2=0x5F3759DF, op0=A.mult, op1=A.add)
                for _ in range(NEWTON_ITERS):
                    yy = scrpool.tile([128, nt], f32, tag="rsq_f")
                    nc.vector.tensor_mul(yy, rn, rn)
                    nc.vector.tensor_mul(yy, yy, ss)
                    nc.vector.tensor_scalar(
                        out=yy, in0=yy, scalar1=-0.5, scalar2=1.5,
                        op0=A.mult, op1=A.add)
                    nc.vector.tensor_mul(rn, rn, yy)
                # mu0 = z[:, :, 0] * rn.  With unnormalized u = mu - e1 the
                # reflection coefficient is 2/(u.u) = 1/(1 - mu0).
                mu0 = scrpool.tile([128, nt], f32, tag="rsq_m")
                z0 = zsb[:, :, 0:1].rearrange("p t o -> p (t o)")
                nc.vector.tensor_mul(mu0, z0, rn)
                nc.vector.tensor_scalar(
                    out=mu0, in0=mu0, scalar1=-1.0, scalar2=1.0,
                    op0=A.mult, op1=A.add)
                nc.vector.reciprocal(out=c2, in_=mu0)
                # u = z * rn (bf16), then u[., ., 0] -= 1  (u = mu - e1)
                for t in range(nt):
                    nc.vector.tensor_scalar(
                        out=u[:, t, :], in0=zsb[:, t, :],
                        scalar1=rn[:, t:t + 1], scalar2=None, op0=A.mult)
                u0 = u[:, :, 0:1].rearrange("p t o -> p (t o)")
                nc.vector.tensor_scalar(
                    out=u0, in0=u0, scalar1=1.0, scalar2=None, op0=A.subtract)

            prep(z2sb, JT, ss2, rn2, c22, u2)
            prep(z1sb, RT, ss1, rn1, c21, u1)

            # ---- Householder application + transpose, per sample ----
            for s in range(S if stage >= 2 else 0):
                e2in = spool.tile([128, JT, 128], bf16, tag="e2in")
                nc.sync.dma_start(out=e2in, in_=E2[s])
                stage2 = spool.tile([128, JT, 128], bf16, tag="stage2")
                dots2 = spool.tile([128, JT], f32, tag="dots2")
                for t in range(JT):
                    scr = scrpool.tile([128, 128], f32, tag="ttscr")
                    # out = (e * c2) .* u, accum = c2 * <e, u>
                    nc.vector.scalar_tensor_tensor(
                        out=scr, in0=e2in[:, t, :], scalar=c22[:, t:t + 1],
                        in1=u2[:, t, :], op0=A.mult, op1=A.mult,
                        accum_out=dots2[:, t:t + 1])
                    w = scrpool.tile([128, 128], bf16, tag="wscr")
                    nc.vector.tensor_scalar(
                        out=w, in0=u2[:, t, :], scalar1=dots2[:, t:t + 1],
                        scalar2=None, op0=A.mult)
                    nc.vector.tensor_sub(stage2[:, t, :], e2in[:, t, :], w)
                nc.sync.dma_start_transpose(
                    out=s2T[:, s, :].rearrange("p (t r) -> p t r", r=128),
                    in_=stage2[:, :, :].rearrange("p t r -> p (t r)"))

                e1in = spool.tile([128, RT, 128], bf16, tag="e1in")
                nc.sync.dma_start(out=e1in, in_=E1[s])
                stage1 = spool.tile([128, RT, 128], bf16, tag="stage1")
                dots1 = spool.tile([128, RT], f32, tag="dots1")
                for t in range(RT):
                    scr = scrpool.tile([128, 128], f32, tag="ttscr")
                    nc.vector.scalar_tensor_tensor(
                        out=scr, in0=e1in[:, t, :], scalar=c21[:, t:t + 1],
                        in1=u1[:, t, :], op0=A.mult, op1=A.mult,
                        accum_out=dots1[:, t:t + 1])
                    w = scrpool.tile([128, 128], bf16, tag="wscr")
                    nc.vector.tensor_scalar(
                        out=w, in0=u1[:, t, :], scalar1=dots1[:, t:t + 1],
                        scalar2=None, op0=A.mult)
                    nc.vector.tensor_sub(stage1[:, t, :], e1in[:, t, :], w)
                nc.sync.dma_start_transpose(
                    out=s1T[:, s, :].rearrange("p (t r) -> p t r", r=128),
                    in_=stage1[:, :, :].rearrange("p t r -> p (t r)"))

            # ---- scores + exp/accumulate ----
            nc.vector.memset(SUMS2[:], 1.0)
            nc.vector.memset(DIAG[:], 0.0)
            for s in range(S if stage >= 3 else 0):
                for mt in range(RT):
                    lhsT = s1T[:, s, mt * 128:(mt + 1) * 128]
                    for k in range(NCHUNK):
                        chunk = ppool.tile([128, CHUNK], f32, tag="chunk")
                        for n in range(CHUNK // 512):
                            j0 = k * CHUNK + n * 512
                            nc.tensor.matmul(
                                chunk[:, n * 512:(n + 1) * 512],
                                lhsT=lhsT,
                                rhs=s2T[:, s, j0:j0 + 512],
                                start=True, stop=True)
                        if k == 0:
                            scrd = scrpool.tile([128, 128], f32, tag="dgscr")
                            nc.vector.scalar_tensor_tensor(
                                out=scrd,
                                in0=chunk[:, mt * 128:(mt + 1) * 128],
                                scalar=1.0, in1=iden,
                                op0=A.bypass, op1=A.mult,
                                accum_out=DIAG[:, mt * S + s:mt * S + s + 1])
                        idx = (mt * S + s) * NCHUNK + k
                        if stage >= 4:
                            nc.scalar.activation(
                                out=chunk[:, :], in_=chunk[:, :], func=AF.Exp,
                                bias=bm_shift[:], scale=K_POS,
                                accum_out=SUMS2[:, idx:idx + 1])
                        else:
                            nc.vector.tensor_reduce(
                                out=SUMS2[:, idx:idx + 1],
                                in_=chunk[:, :], axis=AX.X, op=A.max)

            # ---- tail ----
            nc.vector.tensor_reduce(
                out=SUMS, in_=SUMS2[:].rearrange("p (g k) -> p g k", k=NCHUNK),
                axis=AX.X, op=A.add)
            nc.scalar.activation(out=LSE, in_=SUMS, func=AF.Ln)
            nc.vector.tensor_scalar(
                out=RATIO, in0=DIAG, scalar1=K_POS, scalar2=(LNB - SHIFT),
                op0=A.mult, op1=A.add)
            nc.vector.tensor_sub(RATIO, RATIO, LSE)
            nc.scalar.activation(out=TEXP, in_=RATIO, func=AF.Exp,
                                 bias=bm_lnb[:])
            nc.vector.tensor_reduce(
                out=T4, in_=TEXP[:].rearrange("p (t s) -> p t s", s=S),
                axis=AX.X, op=A.add)
            nc.scalar.activation(out=LG, in_=T4, func=AF.Ln)
            nc.vector.tensor_scalar(
                out=LI, in0=LG, scalar1=-1.0, scalar2=(LNS - LNB),
                op0=A.mult, op1=A.add)
            nc.vector.tensor_reduce(out=LIC, in_=LI, axis=AX.X, op=A.add)
            nc.sync.dma_start(out=OUT[:], in_=LIC[:])

    nc.finalize()
    _cache[key] = nc
    return nc


# --------------------------------------------------------------------------
# Entry point
# --------------------------------------------------------------------------

def _in_maps(z1, z2):
    e1_percore, e2_percore, iden = _host_consts()
    maps = []
    for c in range(NCORES):
        z2r = np.roll(z2, -c * BSH, axis=0)
        z2r = np.ascontiguousarray(
            z2r.reshape(JT, 128, 128).transpose(1, 0, 2))
        z1s = np.ascontiguousarray(
            z1[c * BSH:(c + 1) * BSH].reshape(RT, 128, 128).transpose(1, 0, 2))
        maps.append({
            "z2r": z2r, "z1s": z1s,
            "e2": e2_percore[c], "e1": e1_percore[c],
            "iden": iden,
        })
    return maps


def _run(z1, z2, trace=False, stage=4, **trace_kwargs):
    from concourse.bass_utils import run_bass_kernel_spmd
    nc = _build_program(stage)
    maps = _in_maps(z1, z2)
    res = run_bass_kernel_spmd(nc, maps, list(range(NCORES)), trace=trace,
                               **trace_kwargs)
    total = sum(float(np.asarray(r["out"], dtype=np.float64).sum())
                for r in res.results)
    return np.float32(total / B), res


def kernel(z1, z2, n_samples):
    assert int(n_samples) == S, f"kernel compiled for n_samples={S}"
    z1 = np.ascontiguousarray(np.asarray(z1, dtype=np.float32))
    z2 = np.ascontiguousarray(np.asarray(z2, dtype=np.float32))
    out, _ = _run(z1, z2, trace=False)
    return out


# revision 28
# speedup vs baseline: 1.4646x; 1.4646x over previous
"""MCInfoNCE loss on 8 Trainium2 NeuronCores (Bass/Tile).

Sharding: data-parallel over the query axis of the [S, B, B] score tensor.
Each core owns B/8 = 512 query rows and sees all B negative keys
(replicated key side, i.e. the "all-gather" of standard distributed
InfoNCE is realized by handing every core the full key tensors). The key
tensors handed to core c are rotated by c*512 rows so that each core's
own positive diagonal block sits at local column block 0 -- this keeps
the compiled program identical across cores (pure SPMD, no partition id).

The vMF proposal samples around e1 (Wood's accept-reject scheme) depend
only on the fixed RNG key(42), never on the inputs, so they are
reproduced once on the host with jax-CPU and streamed to the device as
constants. Everything input-dependent runs on device: normalization,
Householder reflection, the [S,B,B] score matmul, the
logsumexp-over-keys, the logsumexp-over-samples, and the final mean.

Device pipeline per core:
  1. prep:  sumsq(z) -> rsqrt (Newton on DVE) -> u = z*rn - e1,
            c2 = 2/(1-mu0)   (Householder with unnormalized u)
  2. per sample s: dot = <e, u> (DVE ttr), w = u*dot*c2 (DVE ts),
            srow = e - w (DVE tt, bf16), then one xbar DMA transpose
            into the [D, S*B] key matrix / [D, S*512] query matrix.
  3. scores: PE matmuls (bf16, K=D=128) into [128, 2048] PSUM chunks;
            diagonal extracted via identity ttr; ScalarE computes
            exp(K_POS*x - 20) in place with row-accumulate.
  4. tail:  ln / exp / ln stack on [128, 32] buffers, partition
            all-reduce, one scalar DMA out per core. Host sums 8
            partials and divides by B.
"""

import numpy as np

S, B, D = 8, 4096, 128
NCORES = 8
BSH = B // NCORES          # 512 query rows per core
RT = BSH // 128            # 4 row tiles per core
JT = B // 128              # 32 key tiles
KAPPA, K_POS, ROUNDS = 32.0, 20.0, 8
SHIFT = 20.0               # fixed logsumexp shift (scores <= K_POS = 20)
LNB = float(np.log(B))
LNS = float(np.log(S))
CHUNK = 2048               # PSUM chunk (4 banks); 2 chunks in flight
NCHUNK = B // CHUNK
NEWTON_ITERS = 4

_cache = {}


# --------------------------------------------------------------------------
# Host-side constants: vMF samples around e1 (input independent, fixed seed)
# --------------------------------------------------------------------------

def _e1_samples():
    """Reproduce the reference's vMF sampling up to (but excluding) the
    Householder reflection: returns two [S, B, D] float32 arrays."""
    import jax
    import jax.numpy as jnp

    cpu = jax.devices("cpu")[0]

    def sample(key, n_samples, Bb, Dd):
        m = float(Dd)
        k = KAPPA
        sq = jnp.sqrt(4.0 * k * k + (m - 1.0) ** 2)
        b = (-2.0 * k + sq) / (m - 1.0)
        a = (m - 1.0 + 2.0 * k + sq) / 4.0
        d = 4.0 * a * b / (1.0 + b) - (m - 1.0) * jnp.log(m - 1.0)

        kb, ku, kv = jax.random.split(key, 3)
        eps = jax.random.beta(kb, (m - 1.0) / 2.0, (m - 1.0) / 2.0,
                              (ROUNDS, n_samples, Bb))
        u = jax.random.uniform(ku, (ROUNDS, n_samples, Bb))
        denom = 1.0 - (1.0 - b) * eps
        w_prop = (1.0 - (1.0 + b) * eps) / denom
        t = 2.0 * a * b / denom
        accept = (m - 1.0) * jnp.log(t) - t + d >= jnp.log(u)
        first = jnp.argmax(accept, axis=0)
        w = jnp.take_along_axis(w_prop, first[None], axis=0)[0]

        v = jax.random.normal(kv, (n_samples, Bb, Dd - 1))
        v = v / jnp.linalg.norm(v, axis=-1, keepdims=True)
        z = jnp.concatenate(
            [w[..., None], jnp.sqrt(jnp.clip(1.0 - w * w, 0.0))[..., None] * v],
            axis=-1)
        return np.asarray(z, dtype=np.float32)

    with jax.default_device(cpu):
        ka, kb = jax.random.split(jax.random.key(42))
        z1e = sample(ka, S, B, D)
        z2e = sample(kb, S, B, D)
    return z1e, z2e


def _host_consts():
    """Per-core constant input arrays (cached)."""
    if "consts" in _cache:
        return _cache["consts"]
    import ml_dtypes
    bf16 = ml_dtypes.bfloat16
    z1e, z2e = _e1_samples()
    e1_percore = []
    e2_percore = []
    for c in range(NCORES):
        # queries: own 512 rows, [S, 128, RT, 128]
        e1c = z1e[:, c * BSH:(c + 1) * BSH, :]
        e1c = e1c.reshape(S, RT, 128, 128).transpose(0, 2, 1, 3)
        e1_percore.append(np.ascontiguousarray(e1c).astype(bf16))
        # keys: full set rotated by c*512, [S, 128, JT, 128]
        e2c = np.roll(z2e, -c * BSH, axis=1)
        e2c = e2c.reshape(S, JT, 128, 128).transpose(0, 2, 1, 3)
        e2_percore.append(np.ascontiguousarray(e2c).astype(bf16))
    iden = np.eye(128, dtype=np.float32)
    _cache["consts"] = (e1_percore, e2_percore, iden)
    return _cache["consts"]


# --------------------------------------------------------------------------
# Device program
# --------------------------------------------------------------------------

def _build_program(stage=4):
    """stage: 1=loads+prep, 2=+householder/transpose, 3=+scores/exp, 4=full."""
    key = ("prog", stage)
    if key in _cache:
        return _cache[key]
    import concourse.bass as bass
    from concourse import bacc, mybir
    from concourse.tile import TileContext

    f32 = mybir.dt.float32
    bf16 = mybir.dt.bfloat16
    i32 = mybir.dt.int32
    A = mybir.AluOpType
    AF = mybir.ActivationFunctionType
    AX = mybir.AxisListType

    nc = bacc.Bacc()
    Z2 = nc.declare_dram_parameter("z2r", [128, JT, 128], f32, isOutput=False)
    Z1 = nc.declare_dram_parameter("z1s", [128, RT, 128], f32, isOutput=False)
    E2 = nc.declare_dram_parameter("e2", [S, 128, JT, 128], bf16, isOutput=False)
    E1 = nc.declare_dram_parameter("e1", [S, 128, RT, 128], bf16, isOutput=False)
    IDN = nc.declare_dram_parameter("iden", [128, 128], f32, isOutput=False)
    OUT = nc.declare_dram_parameter("out", [128, 1], f32, isOutput=True)

    with TileContext(nc) as tc:
        with (
            tc.tile_pool(name="consts", bufs=1) as cpool,
            tc.tile_pool(name="stream", bufs=2) as spool,
            tc.tile_pool(name="scr", bufs=4) as scrpool,
            tc.tile_pool(name="psum", bufs=2, space="PSUM") as ppool,
        ):
            z2sb = cpool.tile([128, JT, 128], f32)
            z1sb = cpool.tile([128, RT, 128], f32)
            iden = cpool.tile([128, 128], f32)
            u2 = cpool.tile([128, JT, 128], bf16)
            u1 = cpool.tile([128, RT, 128], bf16)
            s2T = cpool.tile([128, S, B], bf16)
            s1T = cpool.tile([128, S, BSH], bf16)
            ss2 = cpool.tile([128, JT], f32)
            rn2 = cpool.tile([128, JT], f32)
            c22 = cpool.tile([128, JT], f32)
            ss1 = cpool.tile([128, RT], f32)
            rn1 = cpool.tile([128, RT], f32)
            c21 = cpool.tile([128, RT], f32)
            SUMS2 = cpool.tile([128, RT * S * NCHUNK], f32)
            DIAG = cpool.tile([128, RT * S], f32)
            SUMS = cpool.tile([128, RT * S], f32)
            LSE = cpool.tile([128, RT * S], f32)
            RATIO = cpool.tile([128, RT * S], f32)
            TEXP = cpool.tile([128, RT * S], f32)
            T4 = cpool.tile([128, RT], f32)
            LG = cpool.tile([128, RT], f32)
            LI = cpool.tile([128, RT], f32)
            LIC = cpool.tile([128, 1], f32)
            bm_shift = cpool.tile([128, 1], f32)
            bm_lnb = cpool.tile([128, 1], f32)
            nc.vector.memset(bm_shift, -SHIFT)
            nc.vector.memset(bm_lnb, -LNB)

            nc.sync.dma_start(out=z2sb, in_=Z2[:])
            nc.sync.dma_start(out=z1sb, in_=Z1[:])
            nc.sync.dma_start(out=iden, in_=IDN[:])

            # ---- prep: rn = 1/||z||  (Newton rsqrt on DVE), u, c2 ----
            def prep(zsb, nt, ss, rn, c2, u):
                # sum of squares per row tile, batched in halves
                half = max(nt // 2, 1)
                for h in range(0, nt, half):
                    hn = min(half, nt - h)
                    psq = scrpool.tile([128, half * 128], f32, tag="bigf", bufs=2)
                    zv = zsb[:, h:h + hn, :]
                    nc.vector.tensor_mul(
                        psq[:, :hn * 128],
                        zv.rearrange("p t d -> p (t d)"),
                        zv.rearrange("p t d -> p (t d)"))
                    nc.vector.tensor_reduce(
                        out=ss[:, h:h + hn],
                        in_=psq[:, :hn * 128].rearrange(
                            "p (t d) -> p t d", d=128),
                        axis=AX.X, op=A.add)
                # rsqrt seed via int bit trick: y = 0x5f3759df - (bits >> 1)
                tmpi = scrpool.tile([128, nt], i32, tag="rsq_i")
                nc.vector.tensor_scalar(
                    out=tmpi, in0=ss[:].bitcast(i32), scalar1=1, scalar2=None,
                    op0=A.logical_shift_right)
                nc.vector.tensor_scalar(
                    out=rn[:].bitcast(i32), in0=tmpi, scalar1=-1,
                    scalar2=0x5F3759DF, op0=A.mult, op1=A.add)
                for _ in range(NEWTON_ITERS):
                    yy = scrpool.tile([128, nt], f32, tag="rsq_f")
                    nc.vector.tensor_mul(yy, rn, rn)
                    nc.vector.tensor_mul(yy, yy, ss)
                    nc.vector.tensor_scalar(
                        out=yy, in0=yy, scalar1=-0.5, scalar2=1.5,
                        op0=A.mult, op1=A.add)
                    nc.vector.tensor_mul(rn, rn, yy)
                # mu0 = z[:, :, 0] * rn.  With unnormalized u = mu - e1 the
                # reflection coefficient is 2/(u.u) = 1/(1 - mu0).
                mu0 = scrpool.tile([128, nt], f32, tag="rsq_m")
                z0 = zsb[:, :, 0:1].rearrange("p t o -> p (t o)")
                nc.vector.tensor_mul(mu0, z0, rn)
                nc.vector.tensor_scalar(
                    out=mu0, in0=mu0, scalar1=-1.0, scalar2=1.0,
                    op0=A.mult, op1=A.add)
                nc.vector.reciprocal(out=c2, in_=mu0)
                # u = z * rn (bf16, rn broadcast along d), then u[...,0] -= 1
                nc.vector.tensor_mul(
                    u[:, :, :], zsb[:, :, :],
                    rn[:].to_broadcast([128, nt, 128]))
                u0 = u[:, :, 0:1].rearrange("p t o -> p (t o)")
                nc.vector.tensor_scalar(
                    out=u0, in0=u0, scalar1=1.0, scalar2=None, op0=A.subtract)

            prep(z2sb, JT, ss2, rn2, c22, u2)
            prep(z1sb, RT, ss1, rn1, c21, u1)

            # ---- Householder application + transpose, per sample ----
            # Batched over all row tiles: 5 big DVE ops + 1 GpSimd sub per
            # side instead of 3 ops per [128,128] tile.
            def householder(ein, u, c2, nt, stage, dots_tag):
                dots = spool.tile([128, nt], f32, tag=dots_tag)
                dotc = spool.tile([128, nt], bf16, tag=dots_tag + "c")
                prod = scrpool.tile([128, nt * 128], bf16, tag="bigP", bufs=2)
                nc.vector.tensor_mul(
                    prod, ein[:].rearrange("p t d -> p (t d)"),
                    u[:].rearrange("p t d -> p (t d)"))
                nc.vector.tensor_reduce(
                    out=dots,
                    in_=prod[:].rearrange("p (t d) -> p t d", d=128),
                    axis=AX.X, op=A.add)
                nc.vector.tensor_mul(dotc, dots, c2)
                w = scrpool.tile([128, nt, 128], bf16, tag="bigW", bufs=2)
                nc.vector.tensor_mul(
                    w, u[:, :, :], dotc[:].to_broadcast([128, nt, 128]))
                nc.gpsimd.tensor_sub(stage[:, :, :], ein[:, :, :], w)

            for s in range(S if stage >= 2 else 0):
                e1in = spool.tile([128, RT, 128], bf16, tag="e1in")
                nc.sync.dma_start(out=e1in, in_=E1[s])
                stage1 = spool.tile([128, RT, 128], bf16, tag="stage1")
                householder(e1in, u1, c21, RT, stage1, "dots1")
                nc.sync.dma_start_transpose(
                    out=s1T[:, s, :].rearrange("p (t r) -> p t r", r=128),
                    in_=stage1[:, :, :].rearrange("p t r -> p (t r)"))

                e2in = spool.tile([128, JT, 128], bf16, tag="e2in")
                nc.sync.dma_start(out=e2in, in_=E2[s])
                stage2 = spool.tile([128, JT, 128], bf16, tag="stage2")
                householder(e2in, u2, c22, JT, stage2, "dots2")
                nc.sync.dma_start_transpose(
                    out=s2T[:, s, :].rearrange("p (t r) -> p t r", r=128),
                    in_=stage2[:, :, :].rearrange("p t r -> p (t r)"))

                # pos diagonal from row space: local key block t pairs with
                # query row tile t (keys are rotated by c*512 per core)
                pd = scrpool.tile([128, RT * 128], bf16, tag="bigPD", bufs=2)
                nc.vector.tensor_mul(
                    pd, stage1[:, :, :].rearrange("p t d -> p (t d)"),
                    stage2[:, 0:RT, :].rearrange("p t d -> p (t d)"))
                nc.vector.tensor_reduce(
                    out=DIAG[:, s * RT:(s + 1) * RT],
                    in_=pd[:].rearrange("p (t d) -> p t d", d=128),
                    axis=AX.X, op=A.add)

            # ---- scores + exp/accumulate ----
            if stage < 4:
                nc.vector.memset(SUMS2[:], 1.0)
            if stage < 2:
                nc.vector.memset(DIAG[:], 0.0)
            for s in range(S if stage >= 3 else 0):
                for mt in range(RT):
                    lhsT = s1T[:, s, mt * 128:(mt + 1) * 128]
                    for k in range(NCHUNK):
                        chunk = ppool.tile([128, CHUNK], f32, tag="chunk")
                        for n in range(CHUNK // 512):
                            j0 = k * CHUNK + n * 512
                            nc.tensor.matmul(
                                chunk[:, n * 512:(n + 1) * 512],
                                lhsT=lhsT,
                                rhs=s2T[:, s, j0:j0 + 512],
                                start=True, stop=True)
                        idx = (s * RT + mt) * NCHUNK + k
                        if stage >= 4:
                            nc.scalar.activation(
                                out=chunk[:, :], in_=chunk[:, :], func=AF.Exp,
                                bias=bm_shift[:], scale=K_POS,
                                accum_out=SUMS2[:, idx:idx + 1])
                        else:
                            nc.vector.tensor_reduce(
                                out=SUMS2[:, idx:idx + 1],
                                in_=chunk[:, :], axis=AX.X, op=A.max)

            # ---- tail ----
            nc.vector.tensor_reduce(
                out=SUMS, in_=SUMS2[:].rearrange("p (g k) -> p g k", k=NCHUNK),
                axis=AX.X, op=A.add)
            nc.scalar.activation(out=LSE, in_=SUMS, func=AF.Ln)
            nc.vector.tensor_scalar(
                out=RATIO, in0=DIAG, scalar1=K_POS, scalar2=(LNB - SHIFT),
                op0=A.mult, op1=A.add)
            nc.vector.tensor_sub(RATIO, RATIO, LSE)
            nc.scalar.activation(out=TEXP, in_=RATIO, func=AF.Exp,
                                 bias=bm_lnb[:])
            nc.vector.tensor_reduce(
                out=T4, in_=TEXP[:].rearrange("p (s t) -> p t s", s=S),
                axis=AX.X, op=A.add)
            nc.scalar.activation(out=LG, in_=T4, func=AF.Ln)
            nc.vector.tensor_scalar(
                out=LI, in0=LG, scalar1=-1.0, scalar2=(LNS - LNB),
                op0=A.mult, op1=A.add)
            nc.vector.tensor_reduce(out=LIC, in_=LI, axis=AX.X, op=A.add)
            nc.sync.dma_start(out=OUT[:], in_=LIC[:])

    nc.finalize()
    _cache[key] = nc
    return nc


# --------------------------------------------------------------------------
# Entry point
# --------------------------------------------------------------------------

def _in_maps(z1, z2):
    e1_percore, e2_percore, iden = _host_consts()
    maps = []
    for c in range(NCORES):
        z2r = np.roll(z2, -c * BSH, axis=0)
        z2r = np.ascontiguousarray(
            z2r.reshape(JT, 128, 128).transpose(1, 0, 2))
        z1s = np.ascontiguousarray(
            z1[c * BSH:(c + 1) * BSH].reshape(RT, 128, 128).transpose(1, 0, 2))
        maps.append({
            "z2r": z2r, "z1s": z1s,
            "e2": e2_percore[c], "e1": e1_percore[c],
            "iden": iden,
        })
    return maps


def _run(z1, z2, trace=False, stage=4, **trace_kwargs):
    from concourse.bass_utils import run_bass_kernel_spmd
    nc = _build_program(stage)
    maps = _in_maps(z1, z2)
    res = run_bass_kernel_spmd(nc, maps, list(range(NCORES)), trace=trace,
                               **trace_kwargs)
    total = sum(float(np.asarray(r["out"], dtype=np.float64).sum())
                for r in res.results)
    return np.float32(total / B), res


def kernel(z1, z2, n_samples):
    assert int(n_samples) == S, f"kernel compiled for n_samples={S}"
    z1 = np.ascontiguousarray(np.asarray(z1, dtype=np.float32))
    z2 = np.ascontiguousarray(np.asarray(z2, dtype=np.float32))
    out, _ = _run(z1, z2, trace=False)
    return out
